# revision 1
# baseline (speedup 1.0000x reference)
"""Trainium2 Bass kernel for a dense attention block.

Reference computation (per batch b):
    qkv = x @ w_qkv                      # [S, 48*128]; cols = [q(16 heads) | k(16) | v(16)]
    per head h: logits = q_h @ k_h^T / sqrt(128); attn = softmax(logits)
    combined = concat_h(attn @ v_h)      # [S, 16*128]
    out = combined @ w_out               # [S, 2048]

Sharding (8 cores): 2-way data-parallel over batch x 4-way tensor-parallel
over heads (4 heads per core). Each core computes a partial out [S, 2048]
(its heads' contribution through w_out); host sums the 4 head-group
partials per batch.

Per-core kernel layout strategy (everything arranged so every matmul is a
[128,128] stationary x [128,512] moving fp32r matmul):
  - host passes xT = x[b].T  [DIM, S]   (dim on partitions)
  - qT_h, kT_h computed transposed [128, S]  (head_dim on partitions)
  - v computed natural, 4 heads side by side: v4 tiles [128 tokens, 512]
  - logitsT tile [sk=128, sq=512] = kT_slice.T @ qT_slice
  - expT = exp(scale * logitsT)  (no max subtraction: logits ~ N(0,1))
  - rowsum via ones-matmul [1, sq] accumulated over sk tiles
  - combinedT_h [d=128, sq] = sum_sk v4_slice.T @ expT, scaled by recip
    (recip broadcast across partitions via a DRAM-bounce DMA)
  - out[tok, :] = sum_h combT_h[:, tok_slice].T @ w_out_rows_h
"""

import numpy as np

B, S, DIM = 2, 2048, 2048
NUM_HEADS, HEAD_DIM = 16, 128
N_CORES = 8
HEAD_SHARDS = 4          # tensor-parallel over head groups
NH = NUM_HEADS // HEAD_SHARDS   # 4 heads per core


def build_nc(S=2048, DIN=2048, NH=4, HD=128, DOUT=2048, FREE=512, use_fp32r=True, phases="ABCD", repeat=1):
    import concourse.bacc as bacc
    import concourse.mybir as mybir
    import concourse.tile as tile
    from contextlib import ExitStack

    fp32 = mybir.dt.float32
    mmdt = mybir.dt.float32r if use_fp32r else mybir.dt.float32
    P = 128
    n_t = DIN // P          # contraction tiles for the projections
    n_s4 = S // FREE        # token slices
    n_sk = S // P           # key tiles
    n_dm = DOUT // FREE
    NQK = 2 * NH            # q,k feature tiles of width HD
    VCOLS = NH * HD         # v block width (all heads side by side)
    assert VCOLS <= 512
    scale = 1.0 / (HD ** 0.5)
    Exp = mybir.ActivationFunctionType.Exp

    nc = bacc.Bacc("TRN2")
    xT = nc.dram_tensor("xT", [DIN, S], mmdt, kind="ExternalInput")
    w_qkv = nc.dram_tensor("w_qkv", [DIN, 3 * NH * HD], mmdt, kind="ExternalInput")
    w_out = nc.dram_tensor("w_out", [NH * HD, DOUT], mmdt, kind="ExternalInput")
    out = nc.dram_tensor("out", [S, DOUT], fp32, kind="ExternalOutput")

    def mm(ps, lhsT, rhs, start, stop):
        nc.tensor.matmul(ps, lhsT, rhs, start=start, stop=stop)

    TC = 4 if n_t % 4 == 0 else (2 if n_t % 2 == 0 else 1)
    n_xc = n_t // TC          # x chunks per token-slice
    with tile.TileContext(nc) as tc, ExitStack() as ctx:
        persist = ctx.enter_context(tc.tile_pool(name="persist", bufs=1))
        ones_f32 = persist.tile([P, 1], fp32, tag="ones_f32")
        nc.vector.memset(ones_f32, 1.0)
        ones = persist.tile([P, 1], mmdt, tag="ones")
        nc.vector.tensor_copy(ones, ones_f32)

        v4pool = ctx.enter_context(tc.tile_pool(name="v4", bufs=1))
        v4 = [v4pool.tile([P, VCOLS], mmdt, tag=f"v4_{i}", name=f"v4_{i}")
              for i in range(n_sk)]

        # ------- Phase V: v projection (natural layout), x streamed ---------
        # qT/kT reserved first so the shared x pool can be released after QK
        qkpool = ctx.enter_context(tc.tile_pool(name="qkT", bufs=1))
        qT = [qkpool.tile([P, S], mmdt, tag=f"qT{h}", name=f"qT{h}")
              for h in range(NH)]
        kT = [qkpool.tile([P, S], mmdt, tag=f"kT{h}", name=f"kT{h}")
              for h in range(NH)]
        xstack = ExitStack()
        xpool = xstack.enter_context(tc.tile_pool(name="xs", bufs=n_xc + 1))

        def load_x_chunks(s4):
            xc = []
            for c in range(n_xc):
                xchunk = xpool.tile([P, TC, FREE], mmdt, tag="x")
                nc.sync.dma_start(
                    out=xchunk,
                    in_=xT[:, s4 * FREE:(s4 + 1) * FREE].rearrange(
                        "(t p) s -> p t s", p=P)[:, c * TC:(c + 1) * TC, :])
                xc.append(xchunk)
            return xc

        with tc.tile_pool(name="wvc", bufs=1) as wvpool, \
             tc.tile_pool(name="psv", bufs=4, space="PSUM") as psv:
            wv_col = wvpool.tile([P, n_t, VCOLS], mmdt, tag="wvc")
            wv_dram = w_qkv[:, NQK * HD:NQK * HD + VCOLS].rearrange(
                "(t p) c -> p t c", p=P)
            xc = load_x_chunks(0)
            for c in range(n_xc):
                nc.sync.dma_start(out=wv_col[:, c * TC:(c + 1) * TC, :],
                                  in_=wv_dram[:, c * TC:(c + 1) * TC, :])
            for rep in range(repeat):
             for s4 in range(n_s4):
                if s4 > 0 or rep > 0:
                    xc = load_x_chunks(s4)
                nsv = FREE // P
                psvs = [psv.tile([P, VCOLS], fp32, tag="psv",
                                 name=f"psv{rep}_{s4}_{i}") for i in range(nsv)]
                for t in range(n_t):
                    for sv in range(nsv):
                        mm(psvs[sv],
                           xc[t // TC][:, t % TC, sv * P:(sv + 1) * P],
                           wv_col[:, t, :], t == 0, t == n_t - 1)
                for sv in range(nsv):
                    nc.vector.tensor_copy(v4[s4 * nsv + sv], psvs[sv])

        # ------- Phase QK: q,k projections (transposed), x streamed again ---
        with tc.tile_pool(name="wqk", bufs=NQK) as wpool, \
             tc.tile_pool(name="psqk", bufs=3, space="PSUM") as psqk:
            wq = [None] * NQK
            def load_wq(f):
                wcol = wpool.tile([P, n_t, HD], mmdt, tag="w", name=f"wq{f}")
                nc.sync.dma_start(
                    out=wcol,
                    in_=w_qkv[:, f * HD:(f + 1) * HD].rearrange(
                        "(t p) c -> p t c", p=P))
                wq[f] = wcol
            load_wq(0)   # rest are loaded after the first x chunks (fill overlap)
            for rep in range(repeat):
             for s4 in range(n_s4):
                xc = load_x_chunks(s4)
                if s4 == 0 and rep == 0:
                    for f in range(1, NQK):
                        load_wq(f)
                for f in range(NQK):
                    ps = psqk.tile([P, FREE], fp32, tag="ps")
                    for t in range(n_t):
                        mm(ps, wq[f][:, t, :], xc[t // TC][:, t % TC, :],
                           t == 0, t == n_t - 1)
                    dst = qT[f] if f < NH else kT[f - NH]
                    nc.vector.tensor_copy(
                        dst[:, s4 * FREE:(s4 + 1) * FREE], ps)
        xstack.close()

        do_c = "C" in phases
        # ---------------- Phase C: attention --------------------------------
        combpool = ctx.enter_context(tc.tile_pool(name="comb", bufs=1))
        comb = [combpool.tile([P, S], mmdt, tag=f"comb{h}", name=f"comb{h}")
                for h in range(NH)]
        # w_out preloaded here so its DMAs overlap attention compute
        wopool = ctx.enter_context(tc.tile_pool(name="wo", bufs=NH))
        wo = []
        for hd in range(NH if do_c and "D" in phases else 0):
            wtile = wopool.tile([P, DOUT], mmdt, tag="wo", name=f"wo{hd}")
            nc.sync.dma_start(out=wtile, in_=w_out[hd * P:(hd + 1) * P, :])
            wo.append(wtile)
        with tc.tile_pool(name="et", bufs=6) as epool, \
             tc.tile_pool(name="small", bufs=3) as spool, \
             tc.tile_pool(name="rbc", bufs=3) as bpool, \
             tc.tile_pool(name="dscratch", bufs=3, space="DRAM") as dpool, \
             tc.tile_pool(name="pslg", bufs=4, space="PSUM") as pslg, \
             tc.tile_pool(name="psav", bufs=2, space="PSUM") as psav, \
             tc.tile_pool(name="psrs", bufs=2, space="PSUM") as psrs:
            for rep in range(repeat):
             for h in range(NH if do_c else 0):
                for sq4 in range(n_s4):
                    sq = slice(sq4 * FREE, (sq4 + 1) * FREE)
                    ps_av = psav.tile([P, FREE], fp32, tag="av")
                    ps_rs = psrs.tile([1, FREE], fp32, tag="rs")
                    for skt in range(n_sk):
                        ps_lg = pslg.tile([P, FREE], fp32, tag="lg")
                        mm(ps_lg, kT[h][:, skt * P:(skt + 1) * P], qT[h][:, sq],
                           True, True)
                        et = epool.tile([P, FREE], mmdt, tag="et")
                        nc.scalar.activation(out=et, in_=ps_lg, func=Exp,
                                             scale=scale)
                        mm(ps_rs, ones, et, skt == 0, skt == n_sk - 1)
                        mm(ps_av, v4[skt][:, h * HD:(h + 1) * HD], et,
                           skt == 0, skt == n_sk - 1)
                    rs_sb = spool.tile([1, FREE], fp32, tag="rs_sb")
                    nc.vector.reciprocal(rs_sb, ps_rs)
                    rs_dram = dpool.tile([1, FREE], fp32, tag="rs_dram")
                    nc.sync.dma_start(out=rs_dram, in_=rs_sb)
                    rbc = bpool.tile([P, FREE], fp32, tag="rbc")
                    nc.sync.dma_start(out=rbc,
                                      in_=rs_dram.to_broadcast((P, FREE)))
                    nc.vector.tensor_mul(comb[h][:, sq], ps_av, rbc)

        do_d = do_c and "D" in phases
        # ---------------- Phase D: output projection -------------------------
        with tc.tile_pool(name="ot", bufs=3) as opool, \
             tc.tile_pool(name="psout", bufs=8, space="PSUM") as psout:
            for rep in range(repeat):
             for tok in range(S // P if do_d else 0):
                tk = slice(tok * P, (tok + 1) * P)
                ot = opool.tile([P, DOUT], fp32, tag="ot")
                for dm in range(n_dm):
                    dms = slice(dm * FREE, (dm + 1) * FREE)
                    pso = psout.tile([P, FREE], fp32, tag="po")
                    for hd in range(NH):
                        mm(pso, comb[hd][:, tk], wo[hd][:, dms],
                           hd == 0, hd == NH - 1)
                    nc.vector.tensor_copy(ot[:, dms], pso)
                nc.sync.dma_start(out[tk, :], ot)

    nc.compile()
    return nc


def make_in_maps(x, w_qkv, w_out):
    """Shard full inputs into 8 per-core input maps."""
    x = np.asarray(x, dtype=np.float32)
    w_qkv = np.asarray(w_qkv, dtype=np.float32)
    w_out = np.asarray(w_out, dtype=np.float32)
    in_maps = []
    for c in range(N_CORES):
        b = c // HEAD_SHARDS
        hg = c % HEAD_SHARDS
        cols = NH * HEAD_DIM              # 512
        q = w_qkv[:, hg * cols:(hg + 1) * cols]
        k = w_qkv[:, NUM_HEADS * HEAD_DIM + hg * cols:
                  NUM_HEADS * HEAD_DIM + (hg + 1) * cols]
        v = w_qkv[:, 2 * NUM_HEADS * HEAD_DIM + hg * cols:
                  2 * NUM_HEADS * HEAD_DIM + (hg + 1) * cols]
        in_maps.append({
            "xT": np.ascontiguousarray(x[b].T),
            "w_qkv": np.ascontiguousarray(np.concatenate([q, k, v], axis=1)),
            "w_out": np.ascontiguousarray(w_out[hg * cols:(hg + 1) * cols, :]),
        })
    return in_maps


def combine_outputs(partials):
    out = np.zeros((B, S, DIM), dtype=np.float32)
    for c in range(N_CORES):
        out[c // HEAD_SHARDS] += partials[c]
    return out


_NC_CACHE = None


def kernel(x, w_qkv, w_out):
    global _NC_CACHE
    from concourse import bass_utils
    if _NC_CACHE is None:
        _NC_CACHE = build_nc()
    in_maps = make_in_maps(x, w_qkv, w_out)
    res = bass_utils.run_bass_kernel_spmd(
        _NC_CACHE, in_maps, core_ids=list(range(N_CORES)))
    return combine_outputs([r["out"] for r in res.results])


if __name__ == "__main__":
    nc = build_nc()
    print("built ok")

